# revision 9
# baseline (speedup 1.0000x reference)
"""Trainium2 Bass kernel for nn_ContinuousOutputGenerator (v3).

Math (per batch element b):
    proj = gelu(states @ W1 + b1) @ W2 + b2                      [N, O]
    w[g, n] = exp(-|g - p_n|^2 / bw)                             [G, N]
    out[g, :] = (sum_n w[g,n] proj[n,:]) / (sum_n w[g,n] + eps)

v3 replaces the dense [G,N] kernel-matrix materialization (the v2
bottleneck: 16.8M DVE outer-product elements/core at 1x mode) with a
separable low-rank expansion of the scaled Gaussian kernel:

    ws[g=(i,j), n] = wxs[i, px_n] * wys[j, py_n],
    wxs[i, p] = exp(-(g_i-p)^2/bw + Mx_i),  Mx_i = dist(g_i,[0,1])^2/bw
    wxs[i, p] ~= sum_k Phi[i,k] T_k(2p-1)       (Chebyshev fit, exact to 1e-11)
    ws[g, n]  ~= sum_m Phi2[g, m] psi2[m, n],   m over an SVD-compressed
                 rank-R basis of the (k1,k2) product space (R=256).

so pooling becomes two dense GEMMs with NO elementwise kernel build:
    T   = psi2 @ [proj | 1]        (stage 1, bf16 x bf16 -> f32 PSUM)
    num = Phi2 @ T                 (stage 2, f32r x f32r; f32 keeps the
                                    corner-grid cancellation exact)
    out = num[:, :256] / (num[:, 256] + eps * e^{Mx_i+My_j})

The ones-column denominator shares psi2's quantization error with the
numerator (consistent weighted average); Phi2/T stay f32 because bf16
noise there is amplified ~100x by corner-row cancellation (measured).
b2 is folded on the host via sout (S+eps per grid point), like v2.

Engine budget per core (measured primitives): PE ~42us (MM1 64x216ns,
MM2 128x108, stage1 64x108, stage2 64x~120), ACT ~34us (gelu + half the
evacs), DVE ~20us (recip + half the evacs). v2 was 198us.

Sharding: data-parallel over batch, 8 batch elements -> 8 cores.
"""

import sys
from contextlib import ExitStack

import numpy as np

if "/opt/trn_rl_repo" not in sys.path:
    sys.path.insert(0, "/opt/trn_rl_repo")

import ml_dtypes  # noqa: E402

import concourse.bass as bass  # noqa: E402
import concourse.tile as tile  # noqa: E402
from concourse import bacc, bass_utils, mybir  # noqa: E402

F32 = mybir.dt.float32
F32R = mybir.dt.float32r
BF16 = mybir.dt.bfloat16
AF = mybir.ActivationFunctionType

# Problem shape (hardcoded per contract)
B, N, D, H, O = 8, 4096, 256, 512, 256
GRID = 64
G = GRID * GRID
NT = N // 128          # 32 n-tiles
NCHUNK = 8             # MLP processes n in chunks of 512
CSUB = 4               # 128-row subtiles per chunk
GT = G // 128          # 32 g-tiles
BW = 0.1
EPS = 1e-8
KCH = 24               # Chebyshev degree per axis
R = 128                # SVD-compressed product-basis rank
MT = R // 128          # m-tiles
OE = O + 4             # proj + ones col + zero pad (f32r matmul needs even/aligned free dim)


def _body(tc, aps, out_ap):
    nc = tc.nc
    with ExitStack() as ctx:
        # ---------------- persistent SBUF ----------------
        const = ctx.enter_context(tc.tile_pool(name="const", bufs=1))
        w1 = [const.tile([128, H], BF16, tag=f"w1_{k}", name=f"w1_{k}") for k in range(2)]
        w2 = [const.tile([128, O], BF16, tag=f"w2_{k}", name=f"w2_{k}") for k in range(4)]
        b1_sb = const.tile([128, 4], F32, tag="b1")
        invcg_sb = const.tile([128, GT], F32, tag="invcg")
        psiT = const.tile([128, NT * R], BF16, tag="psiT")
        phiT = [
            const.tile([128, G], F32, tag=f"phiT{m}", name=f"phiT{m}")
            for m in range(MT)
        ]
        tsb = [
            const.tile([128, OE], F32, tag=f"tsb{m}", name=f"tsb{m}")
            for m in range(MT)
        ]
        ssb = const.tile([128, GT], F32, tag="ssb")
        rt_sb = const.tile([128, GT], F32, tag="rt_sb")

        # ---------------- const DMAs ----------------
        for k in range(2):
            nc.scalar.dma_start(w1[k][:], aps["W1"][k * 128 : (k + 1) * 128, :])
        for k in range(4):
            nc.scalar.dma_start(w2[k][:], aps["W2"][k * 128 : (k + 1) * 128, :])
        nc.scalar.dma_start(b1_sb[:], aps["b1"].rearrange("(m p) -> p m", p=128))
        nc.scalar.dma_start(invcg_sb[:], aps["invcg"][:])
        # psi2T / phi2T stream on the GPSIMD DMA queue so they don't delay
        # the sync-queue statesT chunks that gate MM1 startup
        for a in range(NT):
            nc.gpsimd.dma_start(
                psiT[:, a * R : (a + 1) * R], aps["psiT"][a * 128 : (a + 1) * 128, :]
            )
        for m in range(MT):
            for q in range(4):
                c0, c1 = q * 1024, (q + 1) * 1024
                nc.gpsimd.dma_start(
                    phiT[m][:, c0:c1], aps["phiT"][m * 128 : (m + 1) * 128, c0:c1]
                )

        # ---------------- streaming pools ----------------
        stp = ctx.enter_context(tc.tile_pool(name="stp", bufs=4))
        hT = ctx.enter_context(tc.tile_pool(name="hT", bufs=2))
        pjp = ctx.enter_context(tc.tile_pool(name="pjp", bufs=6))
        osbp = ctx.enter_context(tc.tile_pool(name="osbp", bufs=6))

        with (
            tc.tile_pool(name="ps_h", bufs=2, space="PSUM") as ps_h,
            tc.tile_pool(name="ps_p", bufs=2, space="PSUM") as ps_p,
            tc.tile_pool(name="ps_t", bufs=1, space="PSUM") as ps_t,
        ):
            # full-bank [128,512] tiles: matmul start=True zeroes the whole
            # PSUM bank, so each accumulator must own its bank exclusively
            tps = [
                ps_t.tile([128, 512], F32, tag=f"tps{m}", name=f"tps{m}")
                for m in range(MT)
            ]

            # ---- phase 1: MLP chunks + stage-1 accumulation ----
            for c in range(NCHUNK):
                sT = [
                    stp.tile([128, 512], BF16, tag=f"sT{k}", name=f"sT{k}")
                    for k in range(2)
                ]
                n0 = c * 512
                for k in range(2):
                    nc.sync.dma_start(
                        sT[k][:], aps["statesT"][k * 128 : (k + 1) * 128, n0 : n0 + 512]
                    )

                # MM1 + exact GELU: hts[m] = gelu(W1^T sT + b1), [h=512, n=512]
                hts = [
                    hT.tile([128, 512], BF16, tag=f"hT{m}", name=f"hT{m}")
                    for m in range(4)
                ]
                for m in range(4):
                    ph = ps_h.tile([128, 512], F32, tag="ph")
                    for k in range(2):
                        nc.tensor.matmul(
                            ph[:],
                            w1[k][:, m * 128 : (m + 1) * 128],
                            sT[k][:],
                            start=(k == 0),
                            stop=(k == 1),
                        )
                    nc.scalar.activation(
                        hts[m][:], ph[:], AF.Gelu, bias=b1_sb[:, m : m + 1]
                    )

                # MM2 -> projext tiles; stage-1 matmuls consume them right away
                for s in range(CSUB):
                    a = c * CSUB + s
                    pj = pjp.tile([128, OE], BF16, tag="pj")
                    pp = ps_p.tile([128, 512], F32, tag="pp")
                    for k in range(4):
                        nc.tensor.matmul(
                            pp[:, :O],
                            hts[k][:, s * 128 : (s + 1) * 128],
                            w2[k][:],
                            start=(k == 0),
                            stop=(k == 3),
                        )
                    nc.vector.tensor_copy(pj[:, :O], pp[:, :O])
                    nc.vector.memset(pj[:, O:OE], 1.0)
                    # stage 1: T[m,:] += psi2T_a[:,m128]^T @ projext_a
                    for m in range(MT):
                        nc.tensor.matmul(
                            tps[m][:, :OE],
                            psiT[:, a * R + m * 128 : a * R + (m + 1) * 128],
                            pj[:],
                            start=(a == 0),
                            stop=(a == NT - 1),
                        )

            # ---- T evac (f32) ----
            for m in range(MT):
                nc.scalar.copy(tsb[m][:], tps[m][:, :OE])

        # ---- phase 2: stage-2 per g-tile ----
        with tc.tile_pool(name="ps_g", bufs=6, space="PSUM") as ps_g:
            for t in range(GT):
                gps = ps_g.tile([128, 512], F32, tag="gps")
                for m in range(MT):
                    nc.tensor.matmul(
                        gps[:, :OE],
                        phiT[m][:, t * 128 : (t + 1) * 128],
                        tsb[m][:],
                        start=(m == 0),
                        stop=(m == MT - 1),
                    )
                # splus = S + eps_g ; r = 1/splus  (per-partition column)
                nc.vector.tensor_add(
                    ssb[:, t : t + 1], gps[:, O : O + 1], invcg_sb[:, t : t + 1]
                )
                nc.vector.reciprocal(rt_sb[:, t : t + 1], ssb[:, t : t + 1])
                osb = osbp.tile([128, O], F32, tag="osb")
                nc.vector.tensor_scalar_mul(osb[:], gps[:, :O], rt_sb[:, t : t + 1])
                eng = nc.gpsimd if t % 2 == 0 else nc.sync
                eng.dma_start(out_ap[t * 128 : (t + 1) * 128, :], osb[:])
            nc.sync.dma_start(aps["sout"][:], ssb[:])


def build_module():
    nc = bacc.Bacc("TRN2", target_bir_lowering=False, debug=False, num_devices=B)
    aps = {
        "statesT": nc.dram_tensor("statesT", (D, N), BF16, kind="ExternalInput").ap(),
        "W1": nc.dram_tensor("W1", (D, H), BF16, kind="ExternalInput").ap(),
        "b1": nc.dram_tensor("b1", (H,), F32, kind="ExternalInput").ap(),
        "W2": nc.dram_tensor("W2", (H, O), BF16, kind="ExternalInput").ap(),
        "psiT": nc.dram_tensor("psiT", (N, R), BF16, kind="ExternalInput").ap(),
        "phiT": nc.dram_tensor("phiT", (R, G), F32, kind="ExternalInput").ap(),
        "invcg": nc.dram_tensor("invcg", (128, GT), F32, kind="ExternalInput").ap(),
        "sout": nc.dram_tensor("sout", (128, GT), F32, kind="ExternalOutput").ap(),
    }
    out_ap = nc.dram_tensor("out", (G, O), F32, kind="ExternalOutput").ap()
    with tile.TileContext(nc) as tc:
        _body(tc, aps, out_ap)
    nc.compile()
    return nc


_NC = None
_BASIS = None


def _get_nc():
    global _NC
    if _NC is None:
        _NC = build_module()
    return _NC


def _host_basis():
    """Grid-only precompute (cached): Chebyshev fit of the scaled 1D kernel
    rows + SVD compression of the (k1,k2) product basis to rank R."""
    global _BASIS
    if _BASIS is not None:
        return _BASIS
    g = np.linspace(-1.0, 1.0, GRID)
    distg = np.maximum(np.maximum(-g, g - 1.0), 0.0)
    M = (distg**2 / BW).astype(np.float64)
    P = 4001
    p = np.linspace(0.0, 1.0, P)
    W = np.exp(-((g[:, None] - p[None, :]) ** 2) / BW + M[:, None])
    V = np.polynomial.chebyshev.chebvander(2 * p - 1, KCH - 1)
    Phi = np.linalg.lstsq(V, W.T, rcond=None)[0].T  # [64, K]
    Phi2full = (Phi[:, None, :, None] * Phi[None, :, None, :]).reshape(G, KCH * KCH)
    U, s, Vt = np.linalg.svd(Phi2full, full_matrices=False)
    Phi2 = np.ascontiguousarray(U[:, :R]).astype(np.float32)      # [G, R]
    SV = np.ascontiguousarray(s[:R, None] * Vt[:R]).astype(np.float32)  # [R, K^2]
    Mg = (M[:, None] + M[None, :]).ravel()
    eps_g = (EPS * np.exp(Mg)).astype(np.float32)  # [G]
    _BASIS = (Phi2, SV, eps_g)
    return _BASIS


def make_in_maps(inputs):
    states = np.asarray(inputs["entity_states"], np.float32)
    pos = np.asarray(inputs["entity_positions"], np.float32)
    W1 = np.asarray(inputs["W1"], np.float32)
    b1 = np.ascontiguousarray(np.asarray(inputs["b1"], np.float32))
    W2 = np.asarray(inputs["W2"], np.float32)

    Phi2, SV, eps_g = _host_basis()
    bf = ml_dtypes.bfloat16
    statesT = np.ascontiguousarray(states.transpose(0, 2, 1)).astype(bf)  # [B, D, N]
    W1b = np.ascontiguousarray(W1).astype(bf)
    W2b = np.ascontiguousarray(W2).astype(bf)
    phiT = np.ascontiguousarray(Phi2.T)  # [R, G] f32
    # invcg in g-tile layout: col t = eps_g for g rows t*128..(t+1)*128
    invcg_t = np.ascontiguousarray(eps_g.reshape(GT, 128).T)  # [128, GT]

    # per-batch Chebyshev product features, SVD-projected: psi2 = SV @ (Tx (x) Ty)
    Vx = np.polynomial.chebyshev.chebvander(2 * pos[..., 0] - 1, KCH - 1)  # [B,N,K]
    Vy = np.polynomial.chebyshev.chebvander(2 * pos[..., 1] - 1, KCH - 1)
    full = (Vx[:, :, :, None] * Vy[:, :, None, :]).reshape(B, N, KCH * KCH)
    psi2 = np.einsum("rk,bnk->bnr", SV, full.astype(np.float32))  # [B, N, R]
    psiT = np.ascontiguousarray(psi2).astype(bf)  # [B, N, R]

    return [
        {
            "statesT": statesT[b],
            "W1": W1b,
            "b1": b1,
            "W2": W2b,
            "psiT": psiT[b],
            "phiT": phiT,
            "invcg": invcg_t,
        }
        for b in range(B)
    ]


def run(inputs, trace=False, **kw):
    nc = _get_nc()
    res = bass_utils.run_bass_kernel_spmd(
        nc, make_in_maps(inputs), core_ids=list(range(B)), trace=trace, **kw
    )
    out = np.stack([r["out"] for r in res.results], axis=0)  # [B, G, O]
    # host fold of b2: out += b2 * frac,  frac = S/(S+eps) = 1 - eps/splus
    b2 = np.asarray(inputs["b2"], np.float32)
    if np.any(b2):
        _, _, eps_g = _host_basis()
        splus = np.stack(
            [r["sout"].T.ravel() for r in res.results], axis=0
        )  # [B, G]
        frac = 1.0 - eps_g[None, :] / splus
        out = out + b2[None, None, :] * frac[:, :, None]
    return out, res


def kernel(**inputs) -> np.ndarray:
    out, _ = run(inputs, trace=False)
    return out


# revision 11
# speedup vs baseline: 1.0242x; 1.0242x over previous
"""Trainium2 Bass kernel for nn_ContinuousOutputGenerator (v3).

Math (per batch element b):
    proj = gelu(states @ W1 + b1) @ W2 + b2                      [N, O]
    w[g, n] = exp(-|g - p_n|^2 / bw)                             [G, N]
    out[g, :] = (sum_n w[g,n] proj[n,:]) / (sum_n w[g,n] + eps)

v3 replaces the dense [G,N] kernel-matrix materialization (the v2
bottleneck: 16.8M DVE outer-product elements/core at 1x mode) with a
separable low-rank expansion of the scaled Gaussian kernel:

    ws[g=(i,j), n] = wxs[i, px_n] * wys[j, py_n],
    wxs[i, p] = exp(-(g_i-p)^2/bw + Mx_i),  Mx_i = dist(g_i,[0,1])^2/bw
    wxs[i, p] ~= sum_k Phi[i,k] T_k(2p-1)       (Chebyshev fit, exact to 1e-11)
    ws[g, n]  ~= sum_m Phi2[g, m] psi2[m, n],   m over an SVD-compressed
                 rank-R basis of the (k1,k2) product space (R=256).

so pooling becomes two dense GEMMs with NO elementwise kernel build:
    T   = psi2 @ [proj | 1]        (stage 1, bf16 x bf16 -> f32 PSUM)
    num = Phi2 @ T                 (stage 2, f32r x f32r; f32 keeps the
                                    corner-grid cancellation exact)
    out = num[:, :256] / (num[:, 256] + eps * e^{Mx_i+My_j})

The ones-column denominator shares psi2's quantization error with the
numerator (consistent weighted average); Phi2/T stay f32 because bf16
noise there is amplified ~100x by corner-row cancellation (measured).
b2 is folded on the host via sout (S+eps per grid point), like v2.

Engine budget per core (measured primitives): PE ~42us (MM1 64x216ns,
MM2 128x108, stage1 64x108, stage2 64x~120), ACT ~34us (gelu + half the
evacs), DVE ~20us (recip + half the evacs). v2 was 198us.

Sharding: data-parallel over batch, 8 batch elements -> 8 cores.
"""

import sys
from contextlib import ExitStack

import numpy as np

if "/opt/trn_rl_repo" not in sys.path:
    sys.path.insert(0, "/opt/trn_rl_repo")

import ml_dtypes  # noqa: E402

import concourse.bass as bass  # noqa: E402
import concourse.tile as tile  # noqa: E402
from concourse import bacc, bass_utils, mybir  # noqa: E402

F32 = mybir.dt.float32
F32R = mybir.dt.float32r
BF16 = mybir.dt.bfloat16
AF = mybir.ActivationFunctionType

# Problem shape (hardcoded per contract)
B, N, D, H, O = 8, 4096, 256, 512, 256
GRID = 64
G = GRID * GRID
NT = N // 128          # 32 n-tiles
NCHUNK = 8             # MLP processes n in chunks of 512
CSUB = 4               # 128-row subtiles per chunk
GT = G // 128          # 32 g-tiles
BW = 0.1
EPS = 1e-8
KCH = 24               # Chebyshev degree per axis
R = 128                # SVD-compressed product-basis rank
MT = R // 128          # m-tiles
OE = O + 4             # proj + ones col + zero pad (f32r matmul needs even/aligned free dim)


def _body(tc, aps, out_ap):
    nc = tc.nc
    with ExitStack() as ctx:
        # ---------------- persistent SBUF ----------------
        const = ctx.enter_context(tc.tile_pool(name="const", bufs=1))
        w1 = [const.tile([128, H], BF16, tag=f"w1_{k}", name=f"w1_{k}") for k in range(2)]
        w2 = [const.tile([128, O], BF16, tag=f"w2_{k}", name=f"w2_{k}") for k in range(4)]
        b1_sb = const.tile([128, 4], F32, tag="b1")
        invcg_sb = const.tile([128, GT], F32, tag="invcg")
        psiT = const.tile([128, NT * R], BF16, tag="psiT")
        phiH = const.tile([128, G], BF16, tag="phiH")
        phiL = const.tile([128, G], BF16, tag="phiL")
        tsbh = const.tile([128, OE], BF16, tag="tsbh")
        tsbl = const.tile([128, OE], BF16, tag="tsbl")
        ssb = const.tile([128, GT], F32, tag="ssb")
        rt_sb = const.tile([128, GT], F32, tag="rt_sb")
        warm = const.tile([128, 2], F32, tag="warm")

        # gelu ACT-table load happens on first use; warm it during the
        # engine-init preamble so chunk-0 gelu isn't gated by it
        nc.vector.memset(warm[:, 0:1], 0.0)
        nc.scalar.activation(warm[:, 1:2], warm[:, 0:1], AF.Gelu)

        # ---------------- const DMAs ----------------
        # scalar queue: only what MM1/gelu need first
        for k in range(2):
            nc.scalar.dma_start(w1[k][:], aps["W1"][k * 128 : (k + 1) * 128, :])
        nc.scalar.dma_start(b1_sb[:], aps["b1"].rearrange("(m p) -> p m", p=128))
        # gpsimd queue: w2 (needed ~15us), psi2T (stage 1), phi2T/invcg (stage 2)
        for k in range(4):
            nc.gpsimd.dma_start(w2[k][:], aps["W2"][k * 128 : (k + 1) * 128, :])
        for a in range(NT):
            nc.gpsimd.dma_start(
                psiT[:, a * R : (a + 1) * R], aps["psiT"][a * 128 : (a + 1) * 128, :]
            )
        nc.gpsimd.dma_start(invcg_sb[:], aps["invcg"][:])
        for q in range(4):
            c0, c1 = q * 1024, (q + 1) * 1024
            nc.gpsimd.dma_start(phiH[:, c0:c1], aps["phiH"][:, c0:c1])
            nc.gpsimd.dma_start(phiL[:, c0:c1], aps["phiL"][:, c0:c1])

        # ---------------- streaming pools ----------------
        stp = ctx.enter_context(tc.tile_pool(name="stp", bufs=4))
        hT = ctx.enter_context(tc.tile_pool(name="hT", bufs=2))
        pjp = ctx.enter_context(tc.tile_pool(name="pjp", bufs=6))
        osbp = ctx.enter_context(tc.tile_pool(name="osbp", bufs=6))

        with (
            tc.tile_pool(name="ps_h", bufs=3, space="PSUM") as ps_h,
            tc.tile_pool(name="ps_p", bufs=2, space="PSUM") as ps_p,
            tc.tile_pool(name="ps_t", bufs=1, space="PSUM") as ps_t,
        ):
            # full-bank [128,512] tiles: matmul start=True zeroes the whole
            # PSUM bank, so each accumulator must own its bank exclusively
            tps = ps_t.tile([128, 512], F32, tag="tps", name="tps")
            hts_of = {}

            def mm1(c):
                """DMA states chunk, MM1, gelu -> hts (bf16)."""
                sT = [
                    stp.tile([128, 512], BF16, tag=f"sT{k}", name=f"sT{k}")
                    for k in range(2)
                ]
                n0 = c * 512
                for k in range(2):
                    nc.sync.dma_start(
                        sT[k][:], aps["statesT"][k * 128 : (k + 1) * 128, n0 : n0 + 512]
                    )
                hts = [
                    hT.tile([128, 512], BF16, tag=f"hT{m}", name=f"hT{m}")
                    for m in range(4)
                ]
                for m in range(4):
                    ph = ps_h.tile([128, 512], F32, tag="ph")
                    for k in range(2):
                        nc.tensor.matmul(
                            ph[:],
                            w1[k][:, m * 128 : (m + 1) * 128],
                            sT[k][:],
                            start=(k == 0),
                            stop=(k == 1),
                        )
                    nc.scalar.activation(
                        hts[m][:], ph[:], AF.Gelu, bias=b1_sb[:, m : m + 1]
                    )
                hts_of[c] = hts

            def mm2(c):
                """MM2 -> projext tiles (DVE evac), then stage-1 matmuls."""
                hts = hts_of.pop(c)
                for s in range(CSUB):
                    a = c * CSUB + s
                    pj = pjp.tile([128, OE], BF16, tag="pj")
                    pp = ps_p.tile([128, 512], F32, tag="pp")
                    for k in range(4):
                        nc.tensor.matmul(
                            pp[:, :O],
                            hts[k][:, s * 128 : (s + 1) * 128],
                            w2[k][:],
                            start=(k == 0),
                            stop=(k == 3),
                        )
                    nc.vector.tensor_copy(pj[:, :O], pp[:, :O])
                    nc.vector.memset(pj[:, O:OE], 1.0)
                    nc.tensor.matmul(
                        tps[:, :OE],
                        psiT[:, a * R : a * R + 128],
                        pj[:],
                        start=(a == 0),
                        stop=(a == NT - 1),
                    )

            # software pipeline: MM1(c+1) issues before MM2(c) so the PE
            # never head-of-line blocks on gelu(c)
            mm1(0)
            for c in range(1, NCHUNK):
                mm1(c)
                mm2(c - 1)
            mm2(NCHUNK - 1)

            # ---- T evac: hi (bf16) + residual lo (bf16) ----
            nc.scalar.copy(tsbh[:], tps[:, :OE])
            nc.vector.tensor_sub(tsbl[:], tps[:, :OE], tsbh[:])

        # ---- phase 2: stage-2 per g-tile, split-precision bf16 ----
        # num = PhiH@Th + PhiH@Tl + PhiL@Th  (~f32 accuracy, bf16 speed)
        with tc.tile_pool(name="ps_g", bufs=6, space="PSUM") as ps_g:
            for t in range(GT):
                gps = ps_g.tile([128, 512], F32, tag="gps")
                g0 = t * 128
                nc.tensor.matmul(
                    gps[:, :OE], phiH[:, g0 : g0 + 128], tsbh[:], start=True, stop=False
                )
                nc.tensor.matmul(
                    gps[:, :OE], phiH[:, g0 : g0 + 128], tsbl[:], start=False, stop=False
                )
                nc.tensor.matmul(
                    gps[:, :OE], phiL[:, g0 : g0 + 128], tsbh[:], start=False, stop=True
                )
                # splus = S + eps_g ; r = 1/splus  (per-partition column)
                nc.vector.tensor_add(
                    ssb[:, t : t + 1], gps[:, O : O + 1], invcg_sb[:, t : t + 1]
                )
                nc.vector.reciprocal(rt_sb[:, t : t + 1], ssb[:, t : t + 1])
                osb = osbp.tile([128, O], F32, tag="osb")
                nc.vector.tensor_scalar_mul(osb[:], gps[:, :O], rt_sb[:, t : t + 1])
                eng = nc.gpsimd if t % 2 == 0 else nc.sync
                eng.dma_start(out_ap[t * 128 : (t + 1) * 128, :], osb[:])
            nc.sync.dma_start(aps["sout"][:], ssb[:])


def build_module():
    nc = bacc.Bacc("TRN2", target_bir_lowering=False, debug=False, num_devices=B)
    aps = {
        "statesT": nc.dram_tensor("statesT", (D, N), BF16, kind="ExternalInput").ap(),
        "W1": nc.dram_tensor("W1", (D, H), BF16, kind="ExternalInput").ap(),
        "b1": nc.dram_tensor("b1", (H,), F32, kind="ExternalInput").ap(),
        "W2": nc.dram_tensor("W2", (H, O), BF16, kind="ExternalInput").ap(),
        "psiT": nc.dram_tensor("psiT", (N, R), BF16, kind="ExternalInput").ap(),
        "phiH": nc.dram_tensor("phiH", (128, G), BF16, kind="ExternalInput").ap(),
        "phiL": nc.dram_tensor("phiL", (128, G), BF16, kind="ExternalInput").ap(),
        "invcg": nc.dram_tensor("invcg", (128, GT), F32, kind="ExternalInput").ap(),
        "sout": nc.dram_tensor("sout", (128, GT), F32, kind="ExternalOutput").ap(),
    }
    out_ap = nc.dram_tensor("out", (G, O), F32, kind="ExternalOutput").ap()
    with tile.TileContext(nc) as tc:
        _body(tc, aps, out_ap)
    nc.compile()
    return nc


_NC = None
_BASIS = None


def _get_nc():
    global _NC
    if _NC is None:
        _NC = build_module()
    return _NC


def _host_basis():
    """Grid-only precompute (cached): Chebyshev fit of the scaled 1D kernel
    rows + SVD compression of the (k1,k2) product basis to rank R."""
    global _BASIS
    if _BASIS is not None:
        return _BASIS
    g = np.linspace(-1.0, 1.0, GRID)
    distg = np.maximum(np.maximum(-g, g - 1.0), 0.0)
    M = (distg**2 / BW).astype(np.float64)
    P = 4001
    p = np.linspace(0.0, 1.0, P)
    W = np.exp(-((g[:, None] - p[None, :]) ** 2) / BW + M[:, None])
    V = np.polynomial.chebyshev.chebvander(2 * p - 1, KCH - 1)
    Phi = np.linalg.lstsq(V, W.T, rcond=None)[0].T  # [64, K]
    Phi2full = (Phi[:, None, :, None] * Phi[None, :, None, :]).reshape(G, KCH * KCH)
    U, s, Vt = np.linalg.svd(Phi2full, full_matrices=False)
    Phi2 = np.ascontiguousarray(U[:, :R]).astype(np.float32)      # [G, R]
    SV = np.ascontiguousarray(s[:R, None] * Vt[:R]).astype(np.float32)  # [R, K^2]
    Mg = (M[:, None] + M[None, :]).ravel()
    eps_g = (EPS * np.exp(Mg)).astype(np.float32)  # [G]
    _BASIS = (Phi2, SV, eps_g)
    return _BASIS


def make_in_maps(inputs):
    states = np.asarray(inputs["entity_states"], np.float32)
    pos = np.asarray(inputs["entity_positions"], np.float32)
    W1 = np.asarray(inputs["W1"], np.float32)
    b1 = np.ascontiguousarray(np.asarray(inputs["b1"], np.float32))
    W2 = np.asarray(inputs["W2"], np.float32)

    Phi2, SV, eps_g = _host_basis()
    bf = ml_dtypes.bfloat16
    statesT = np.ascontiguousarray(states.transpose(0, 2, 1)).astype(bf)  # [B, D, N]
    W1b = np.ascontiguousarray(W1).astype(bf)
    W2b = np.ascontiguousarray(W2).astype(bf)
    phiT = np.ascontiguousarray(Phi2.T)  # [R, G] f32
    phiH = phiT.astype(bf)
    phiL = (phiT - phiH.astype(np.float32)).astype(bf)
    # invcg in g-tile layout: col t = eps_g for g rows t*128..(t+1)*128
    invcg_t = np.ascontiguousarray(eps_g.reshape(GT, 128).T)  # [128, GT]

    # per-batch Chebyshev product features, SVD-projected: psi2 = SV @ (Tx (x) Ty)
    Vx = np.polynomial.chebyshev.chebvander(2 * pos[..., 0] - 1, KCH - 1)  # [B,N,K]
    Vy = np.polynomial.chebyshev.chebvander(2 * pos[..., 1] - 1, KCH - 1)
    full = (Vx[:, :, :, None] * Vy[:, :, None, :]).reshape(B, N, KCH * KCH)
    psi2 = np.einsum("rk,bnk->bnr", SV, full.astype(np.float32))  # [B, N, R]
    psiT = np.ascontiguousarray(psi2).astype(bf)  # [B, N, R]

    return [
        {
            "statesT": statesT[b],
            "W1": W1b,
            "b1": b1,
            "W2": W2b,
            "psiT": psiT[b],
            "phiH": phiH,
            "phiL": phiL,
            "invcg": invcg_t,
        }
        for b in range(B)
    ]


def run(inputs, trace=False, **kw):
    nc = _get_nc()
    res = bass_utils.run_bass_kernel_spmd(
        nc, make_in_maps(inputs), core_ids=list(range(B)), trace=trace, **kw
    )
    out = np.stack([r["out"] for r in res.results], axis=0)  # [B, G, O]
    # host fold of b2: out += b2 * frac,  frac = S/(S+eps) = 1 - eps/splus
    b2 = np.asarray(inputs["b2"], np.float32)
    if np.any(b2):
        _, _, eps_g = _host_basis()
        splus = np.stack(
            [r["sout"].T.ravel() for r in res.results], axis=0
        )  # [B, G]
        frac = 1.0 - eps_g[None, :] / splus
        out = out + b2[None, None, :] * frac[:, :, None]
    return out, res


def kernel(**inputs) -> np.ndarray:
    out, _ = run(inputs, trace=False)
    return out


# revision 12
# speedup vs baseline: 1.0658x; 1.0407x over previous
"""Trainium2 Bass kernel for nn_ContinuousOutputGenerator (v3).

Math (per batch element b):
    proj = gelu(states @ W1 + b1) @ W2 + b2                      [N, O]
    w[g, n] = exp(-|g - p_n|^2 / bw)                             [G, N]
    out[g, :] = (sum_n w[g,n] proj[n,:]) / (sum_n w[g,n] + eps)

v3 replaces the dense [G,N] kernel-matrix materialization (the v2
bottleneck: 16.8M DVE outer-product elements/core at 1x mode) with a
separable low-rank expansion of the scaled Gaussian kernel:

    ws[g=(i,j), n] = wxs[i, px_n] * wys[j, py_n],
    wxs[i, p] = exp(-(g_i-p)^2/bw + Mx_i),  Mx_i = dist(g_i,[0,1])^2/bw
    wxs[i, p] ~= sum_k Phi[i,k] T_k(2p-1)       (Chebyshev fit, exact to 1e-11)
    ws[g, n]  ~= sum_m Phi2[g, m] psi2[m, n],   m over an SVD-compressed
                 rank-R basis of the (k1,k2) product space (R=256).

so pooling becomes two dense GEMMs with NO elementwise kernel build:
    T   = psi2 @ [proj | 1]        (stage 1, bf16 x bf16 -> f32 PSUM)
    num = Phi2 @ T                 (stage 2, f32r x f32r; f32 keeps the
                                    corner-grid cancellation exact)
    out = num[:, :256] / (num[:, 256] + eps * e^{Mx_i+My_j})

The ones-column denominator shares psi2's quantization error with the
numerator (consistent weighted average); Phi2/T stay f32 because bf16
noise there is amplified ~100x by corner-row cancellation (measured).
b2 is folded on the host via sout (S+eps per grid point), like v2.

Engine budget per core (measured primitives): PE ~42us (MM1 64x216ns,
MM2 128x108, stage1 64x108, stage2 64x~120), ACT ~34us (gelu + half the
evacs), DVE ~20us (recip + half the evacs). v2 was 198us.

Sharding: data-parallel over batch, 8 batch elements -> 8 cores.
"""

import sys
from contextlib import ExitStack

import numpy as np

if "/opt/trn_rl_repo" not in sys.path:
    sys.path.insert(0, "/opt/trn_rl_repo")

import ml_dtypes  # noqa: E402

import concourse.bass as bass  # noqa: E402
import concourse.tile as tile  # noqa: E402
from concourse import bacc, bass_utils, mybir  # noqa: E402

F32 = mybir.dt.float32
F32R = mybir.dt.float32r
BF16 = mybir.dt.bfloat16
AF = mybir.ActivationFunctionType

# Problem shape (hardcoded per contract)
B, N, D, H, O = 8, 4096, 256, 512, 256
GRID = 64
G = GRID * GRID
NT = N // 128          # 32 n-tiles
NCHUNK = 8             # MLP processes n in chunks of 512
CSUB = 4               # 128-row subtiles per chunk
GT = G // 128          # 32 g-tiles
BW = 0.1
EPS = 1e-8
KCH = 24               # Chebyshev degree per axis
R = 128                # SVD-compressed product-basis rank
MT = R // 128          # m-tiles
OE = O + 4             # proj + ones col + zero pad (f32r matmul needs even/aligned free dim)


def _body(tc, aps, out_ap):
    nc = tc.nc
    with ExitStack() as ctx:
        # ---------------- persistent SBUF ----------------
        const = ctx.enter_context(tc.tile_pool(name="const", bufs=1))
        w1 = [const.tile([128, H], BF16, tag=f"w1_{k}", name=f"w1_{k}") for k in range(2)]
        w2 = [const.tile([128, O], BF16, tag=f"w2_{k}", name=f"w2_{k}") for k in range(4)]
        b1_sb = const.tile([128, 4], F32, tag="b1")
        invcg_sb = const.tile([128, GT], F32, tag="invcg")
        psiT = const.tile([128, NT * R], BF16, tag="psiT")
        phiH = const.tile([128, G], BF16, tag="phiH")
        phiL = const.tile([128, G], BF16, tag="phiL")
        tsbh = const.tile([128, OE], BF16, tag="tsbh")
        tsbl = const.tile([128, OE], BF16, tag="tsbl")
        ssb = const.tile([128, GT], F32, tag="ssb")
        rt_sb = const.tile([128, GT], F32, tag="rt_sb")
        warm = const.tile([128, 2], F32, tag="warm")

        # gelu ACT-table load happens on first use; warm it during the
        # engine-init preamble so chunk-0 gelu isn't gated by it
        nc.vector.memset(warm[:, 0:1], 0.0)
        nc.scalar.activation(warm[:, 1:2], warm[:, 0:1], AF.Gelu)

        # ---------------- const DMAs ----------------
        # scalar queue: only what MM1/gelu need first
        for k in range(2):
            nc.scalar.dma_start(w1[k][:], aps["W1"][k * 128 : (k + 1) * 128, :])
        nc.scalar.dma_start(b1_sb[:], aps["b1"].rearrange("(m p) -> p m", p=128))
        # gpsimd queue: w2 (needed ~15us), psi2T (stage 1), phi2T/invcg (stage 2)
        for k in range(4):
            nc.gpsimd.dma_start(w2[k][:], aps["W2"][k * 128 : (k + 1) * 128, :])
        for a in range(NT):
            nc.gpsimd.dma_start(
                psiT[:, a * R : (a + 1) * R], aps["psiT"][a * 128 : (a + 1) * 128, :]
            )
        nc.gpsimd.dma_start(invcg_sb[:], aps["invcg"][:])
        for q in range(4):
            c0, c1 = q * 1024, (q + 1) * 1024
            nc.gpsimd.dma_start(phiH[:, c0:c1], aps["phiH"][:, c0:c1])
            nc.gpsimd.dma_start(phiL[:, c0:c1], aps["phiL"][:, c0:c1])

        # ---------------- streaming pools ----------------
        stp = ctx.enter_context(tc.tile_pool(name="stp", bufs=4))
        hT = ctx.enter_context(tc.tile_pool(name="hT", bufs=2))
        pjp = ctx.enter_context(tc.tile_pool(name="pjp", bufs=6))
        osbp = ctx.enter_context(tc.tile_pool(name="osbp", bufs=6))

        with (
            tc.tile_pool(name="ps_h", bufs=3, space="PSUM") as ps_h,
            tc.tile_pool(name="ps_p", bufs=2, space="PSUM") as ps_p,
            tc.tile_pool(name="ps_t", bufs=1, space="PSUM") as ps_t,
        ):
            # full-bank [128,512] tiles: matmul start=True zeroes the whole
            # PSUM bank, so each accumulator must own its bank exclusively
            tps = ps_t.tile([128, 512], F32, tag="tps", name="tps")
            hts_of = {}

            def mm1(c):
                """DMA states chunk, MM1, gelu -> hts (bf16)."""
                sT = [
                    stp.tile([128, 512], BF16, tag=f"sT{k}", name=f"sT{k}")
                    for k in range(2)
                ]
                n0 = c * 512
                for k in range(2):
                    nc.sync.dma_start(
                        sT[k][:], aps["statesT"][k * 128 : (k + 1) * 128, n0 : n0 + 512]
                    )
                hts = [
                    hT.tile([128, 512], BF16, tag=f"hT{m}", name=f"hT{m}")
                    for m in range(4)
                ]
                for m in range(4):
                    ph = ps_h.tile([128, 512], F32, tag="ph")
                    for k in range(2):
                        nc.tensor.matmul(
                            ph[:],
                            w1[k][:, m * 128 : (m + 1) * 128],
                            sT[k][:],
                            start=(k == 0),
                            stop=(k == 1),
                        )
                    nc.scalar.activation(
                        hts[m][:], ph[:], AF.Gelu, bias=b1_sb[:, m : m + 1]
                    )
                hts_of[c] = hts

            def mm2(c):
                """MM2 -> projext tiles (DVE evac), then stage-1 matmuls."""
                hts = hts_of.pop(c)
                for s in range(CSUB):
                    a = c * CSUB + s
                    pj = pjp.tile([128, OE], BF16, tag="pj")
                    pp = ps_p.tile([128, 512], F32, tag="pp")
                    for k in range(4):
                        nc.tensor.matmul(
                            pp[:, :O],
                            hts[k][:, s * 128 : (s + 1) * 128],
                            w2[k][:],
                            start=(k == 0),
                            stop=(k == 3),
                        )
                    nc.vector.tensor_copy(pj[:, :O], pp[:, :O])
                    nc.vector.memset(pj[:, O:OE], 1.0)
                    nc.tensor.matmul(
                        tps[:, :OE],
                        psiT[:, a * R : a * R + 128],
                        pj[:],
                        start=(a == 0),
                        stop=(a == NT - 1),
                    )

            # software pipeline: MM1(c+1) issues before MM2(c) so the PE
            # never head-of-line blocks on gelu(c)
            mm1(0)
            for c in range(1, NCHUNK):
                mm1(c)
                mm2(c - 1)
            mm2(NCHUNK - 1)

            # ---- T evac: hi (bf16) + residual lo (bf16) ----
            nc.scalar.copy(tsbh[:], tps[:, :OE])
            nc.vector.tensor_sub(tsbl[:], tps[:, :OE], tsbh[:])

        # ---- phase 2: stage-2 per g-tile, split-precision bf16 ----
        # num = PhiH@Th + PhiH@Tl + PhiL@Th  (~f32 accuracy, bf16 speed)
        with tc.tile_pool(name="ps_g", bufs=6, space="PSUM") as ps_g:
            for t in range(GT):
                gps = ps_g.tile([128, 512], F32, tag="gps")
                g0 = t * 128
                nc.tensor.matmul(
                    gps[:, :OE], phiH[:, g0 : g0 + 128], tsbh[:], start=True, stop=False
                )
                nc.tensor.matmul(
                    gps[:, :OE], phiH[:, g0 : g0 + 128], tsbl[:], start=False, stop=False
                )
                nc.tensor.matmul(
                    gps[:, :OE], phiL[:, g0 : g0 + 128], tsbh[:], start=False, stop=True
                )
                # splus = S + eps_g ; r = 1/splus  (per-partition column)
                nc.vector.tensor_add(
                    ssb[:, t : t + 1], gps[:, O : O + 1], invcg_sb[:, t : t + 1]
                )
                nc.vector.reciprocal(rt_sb[:, t : t + 1], ssb[:, t : t + 1])
                osb = osbp.tile([128, O], F32, tag="osb")
                nc.scalar.mul(osb[:], gps[:, :O], rt_sb[:, t : t + 1])
                eng = nc.gpsimd if t % 2 == 0 else nc.sync
                eng.dma_start(out_ap[t * 128 : (t + 1) * 128, :], osb[:])
            nc.sync.dma_start(aps["sout"][:], ssb[:])


def build_module():
    nc = bacc.Bacc("TRN2", target_bir_lowering=False, debug=False, num_devices=B)
    aps = {
        "statesT": nc.dram_tensor("statesT", (D, N), BF16, kind="ExternalInput").ap(),
        "W1": nc.dram_tensor("W1", (D, H), BF16, kind="ExternalInput").ap(),
        "b1": nc.dram_tensor("b1", (H,), F32, kind="ExternalInput").ap(),
        "W2": nc.dram_tensor("W2", (H, O), BF16, kind="ExternalInput").ap(),
        "psiT": nc.dram_tensor("psiT", (N, R), BF16, kind="ExternalInput").ap(),
        "phiH": nc.dram_tensor("phiH", (128, G), BF16, kind="ExternalInput").ap(),
        "phiL": nc.dram_tensor("phiL", (128, G), BF16, kind="ExternalInput").ap(),
        "invcg": nc.dram_tensor("invcg", (128, GT), F32, kind="ExternalInput").ap(),
        "sout": nc.dram_tensor("sout", (128, GT), F32, kind="ExternalOutput").ap(),
    }
    out_ap = nc.dram_tensor("out", (G, O), F32, kind="ExternalOutput").ap()
    with tile.TileContext(nc) as tc:
        _body(tc, aps, out_ap)
    nc.compile()
    return nc


_NC = None
_BASIS = None


def _get_nc():
    global _NC
    if _NC is None:
        _NC = build_module()
    return _NC


def _host_basis():
    """Grid-only precompute (cached): Chebyshev fit of the scaled 1D kernel
    rows + SVD compression of the (k1,k2) product basis to rank R."""
    global _BASIS
    if _BASIS is not None:
        return _BASIS
    g = np.linspace(-1.0, 1.0, GRID)
    distg = np.maximum(np.maximum(-g, g - 1.0), 0.0)
    M = (distg**2 / BW).astype(np.float64)
    P = 4001
    p = np.linspace(0.0, 1.0, P)
    W = np.exp(-((g[:, None] - p[None, :]) ** 2) / BW + M[:, None])
    V = np.polynomial.chebyshev.chebvander(2 * p - 1, KCH - 1)
    Phi = np.linalg.lstsq(V, W.T, rcond=None)[0].T  # [64, K]
    Phi2full = (Phi[:, None, :, None] * Phi[None, :, None, :]).reshape(G, KCH * KCH)
    U, s, Vt = np.linalg.svd(Phi2full, full_matrices=False)
    Phi2 = np.ascontiguousarray(U[:, :R]).astype(np.float32)      # [G, R]
    SV = np.ascontiguousarray(s[:R, None] * Vt[:R]).astype(np.float32)  # [R, K^2]
    Mg = (M[:, None] + M[None, :]).ravel()
    eps_g = (EPS * np.exp(Mg)).astype(np.float32)  # [G]
    _BASIS = (Phi2, SV, eps_g)
    return _BASIS


def make_in_maps(inputs):
    states = np.asarray(inputs["entity_states"], np.float32)
    pos = np.asarray(inputs["entity_positions"], np.float32)
    W1 = np.asarray(inputs["W1"], np.float32)
    b1 = np.ascontiguousarray(np.asarray(inputs["b1"], np.float32))
    W2 = np.asarray(inputs["W2"], np.float32)

    Phi2, SV, eps_g = _host_basis()
    bf = ml_dtypes.bfloat16
    statesT = np.ascontiguousarray(states.transpose(0, 2, 1)).astype(bf)  # [B, D, N]
    W1b = np.ascontiguousarray(W1).astype(bf)
    W2b = np.ascontiguousarray(W2).astype(bf)
    phiT = np.ascontiguousarray(Phi2.T)  # [R, G] f32
    phiH = phiT.astype(bf)
    phiL = (phiT - phiH.astype(np.float32)).astype(bf)
    # invcg in g-tile layout: col t = eps_g for g rows t*128..(t+1)*128
    invcg_t = np.ascontiguousarray(eps_g.reshape(GT, 128).T)  # [128, GT]

    # per-batch Chebyshev product features, SVD-projected: psi2 = SV @ (Tx (x) Ty)
    Vx = np.polynomial.chebyshev.chebvander(2 * pos[..., 0] - 1, KCH - 1)  # [B,N,K]
    Vy = np.polynomial.chebyshev.chebvander(2 * pos[..., 1] - 1, KCH - 1)
    full = (Vx[:, :, :, None] * Vy[:, :, None, :]).reshape(B, N, KCH * KCH)
    psi2 = np.einsum("rk,bnk->bnr", SV, full.astype(np.float32))  # [B, N, R]
    psiT = np.ascontiguousarray(psi2).astype(bf)  # [B, N, R]

    return [
        {
            "statesT": statesT[b],
            "W1": W1b,
            "b1": b1,
            "W2": W2b,
            "psiT": psiT[b],
            "phiH": phiH,
            "phiL": phiL,
            "invcg": invcg_t,
        }
        for b in range(B)
    ]


def run(inputs, trace=False, **kw):
    nc = _get_nc()
    res = bass_utils.run_bass_kernel_spmd(
        nc, make_in_maps(inputs), core_ids=list(range(B)), trace=trace, **kw
    )
    out = np.stack([r["out"] for r in res.results], axis=0)  # [B, G, O]
    # host fold of b2: out += b2 * frac,  frac = S/(S+eps) = 1 - eps/splus
    b2 = np.asarray(inputs["b2"], np.float32)
    if np.any(b2):
        _, _, eps_g = _host_basis()
        splus = np.stack(
            [r["sout"].T.ravel() for r in res.results], axis=0
        )  # [B, G]
        frac = 1.0 - eps_g[None, :] / splus
        out = out + b2[None, None, :] * frac[:, :, None]
    return out, res


def kernel(**inputs) -> np.ndarray:
    out, _ = run(inputs, trace=False)
    return out
